# revision 1
# baseline (speedup 1.0000x reference)
"""Trainium2 Bass kernel for nn_EncoderLayer_88476326298146 (sparse graph attention).

Row-sharded across 8 NeuronCores: core c owns nodes [c*2048, (c+1)*2048) and the
edges targeting them (host-sorted by row into 16 windows of 128 rows, padded to a
fixed per-window count TW). k/v (bf16, [k|v] rows) are replicated via AllGather;
per-edge col features come from dma_gather. Scores use the m=0 softmax (exp is
bounded: score <= max pos_att_bias), segment sums run on the PE via host-built
one-hot matrices, and bvec is folded into Wvec as a 4th row with rel4=[rel,1].
"""
import os
import numpy as np

import concourse.bass as bass
import concourse.bacc as bacc
import concourse.mybir as mybir
import concourse.tile as tile
from concourse.bass_utils import run_bass_kernel_spmd
from concourse.library_config import mlp as mlp_lib
from concourse.masks import make_identity

L, E, SP, C, H, DH, HID = 16384, 131072, 20000, 512, 8, 64, 1024
NCORES = 8
RS = L // NCORES
NW = RS // 128
P = 128
F32 = mybir.dt.float32
BF16 = mybir.dt.bfloat16
I16 = mybir.dt.int16
AF = mybir.ActivationFunctionType
ALU = mybir.AluOpType
AX = mybir.AxisListType

_cache = {}
_SKIP = set(os.environ.get("KSKIP", "").split(","))


def _build(TW):
    NT = TW // P
    nc = bacc.Bacc("TRN2", target_bir_lowering=False, debug=False, num_devices=NCORES)

    x_in = nc.dram_tensor("x_in", [RS, C], F32, kind="ExternalInput")
    w_qkv = nc.dram_tensor("w_qkv", [C, 3 * C], F32, kind="ExternalInput")
    b_qkv = nc.dram_tensor("b_qkv", [P, 3 * C], F32, kind="ExternalInput")
    w_o = nc.dram_tensor("w_o", [C, C], F32, kind="ExternalInput")
    b_o = nc.dram_tensor("b_o", [P, C], F32, kind="ExternalInput")
    w_1 = nc.dram_tensor("w_1", [C, HID], F32, kind="ExternalInput")
    b_1 = nc.dram_tensor("b_1", [P, HID], F32, kind="ExternalInput")
    w_2 = nc.dram_tensor("w_2", [HID, C], F32, kind="ExternalInput")
    b_2 = nc.dram_tensor("b_2", [P, C], F32, kind="ExternalInput")
    w_vec4 = nc.dram_tensor("w_vec4", [32, C], F32, kind="ExternalInput")
    ident_in = nc.dram_tensor("ident_in", [P, P], F32, kind="ExternalInput")
    eps_in = nc.dram_tensor("eps_in", [P, 1], F32, kind="ExternalInput")
    ln_rep = nc.dram_tensor("ln_rep", [4, P, C], F32, kind="ExternalInput")
    eidx = nc.dram_tensor("eidx", [NW, P, TW // 16], I16, kind="ExternalInput")
    geo = nc.dram_tensor("geo", [NW, P, NT, 8], F32, kind="ExternalInput")
    biasA = nc.dram_tensor("biasA", [NW, P, NT, 8], F32, kind="ExternalInput")
    s_e2r = nc.dram_tensor("s_e2r", [NW, P, NT, P], BF16, kind="ExternalInput")
    s_r2e = nc.dram_tensor("s_r2e", [NW, P, NT, P], BF16, kind="ExternalInput")
    y_out = nc.dram_tensor("y_out", [RS, C], F32, kind="ExternalOutput")

    x_t = x_in.ap().rearrange("(m p) n -> p m n", p=P)
    inv_s = 1.0 / float(np.sqrt(DH))

    with tile.TileContext(nc) as tc:
        with tc.tile_pool(name="dram", bufs=1, space="DRAM") as dram, \
             tc.tile_pool(name="const", bufs=1) as const:
            nc.gpsimd.load_library(mlp_lib)

            ident = const.tile([P, P], BF16)
            nc.gpsimd.dma_start(ident[:], ident_in.ap())
            eps_t = const.tile([P, 1], F32)
            nc.sync.dma_start(eps_t[:], eps_in.ap())
            wvec_s = const.tile([32, C], BF16)
            nc.gpsimd.dma_start(wvec_s[:], w_vec4.ap())
            q_sbuf = const.tile([P, NW, C], BF16)
            zt_sbuf = const.tile([P, C // P, RS], BF16)

            kv_shard = dram.tile([RS, 2 * C], BF16)
            if "ag" not in _SKIP:
                kvt = dram.tile([L, 2 * C], BF16, addr_space="Shared")
            else:
                kvt = dram.tile([L, 2 * C], BF16)
            attin_d = dram.tile([RS, C], BF16)
            x2_d = dram.tile([RS, C], F32)
            h_d = dram.tile([RS, HID], BF16)

            # ---------- LN helper ----------
            def ln_phase(src_tiled, g_row, b_row):
                with tc.tile_pool(name="lnp", bufs=2) as wk, \
                     tc.tile_pool(name="lnc", bufs=1) as cst, \
                     tc.tile_pool(name="lntp", bufs=2, space="PSUM") as ptp:
                    gr = cst.tile([P, C], F32, name="lng")
                    nc.sync.dma_start(gr[:], g_row)
                    br = cst.tile([P, C], F32, name="lnb")
                    nc.sync.dma_start(br[:], b_row)
                    for m in range(NW):
                        xt = wk.tile([P, C], F32, tag="ln_x")
                        nc.sync.dma_start(xt[:], src_tiled[:, m, :])
                        sx = wk.tile([P, 1], F32, tag="ln_sx")
                        sq = wk.tile([P, C], F32, tag="ln_sq")
                        sx2 = wk.tile([P, 1], F32, tag="ln_sx2")
                        nc.vector.reduce_sum(sx[:], xt[:], axis=AX.X)
                        nc.scalar.activation(sq[:], xt[:], AF.Square)
                        nc.vector.reduce_sum(sx2[:], sq[:], axis=AX.X)
                        mu = wk.tile([P, 1], F32, tag="ln_mu")
                        nc.vector.tensor_scalar_mul(mu[:], sx[:], 1.0 / C)
                        mu2 = wk.tile([P, 1], F32, tag="ln_mu2")
                        nc.vector.tensor_mul(mu2[:], mu[:], mu[:])
                        var = wk.tile([P, 1], F32, tag="ln_var")
                        nc.vector.scalar_tensor_tensor(
                            out=var[:], in0=sx2[:], scalar=1.0 / C, in1=mu2[:],
                            op0=ALU.mult, op1=ALU.subtract)
                        sd = wk.tile([P, 1], F32, tag="ln_sd")
                        nc.scalar.activation(sd[:], var[:], AF.Sqrt, bias=eps_t[:], scale=1.0)
                        rs_ = wk.tile([P, 1], F32, tag="ln_rs")
                        nc.vector.reciprocal(rs_[:], sd[:])
                        nmr = wk.tile([P, 1], F32, tag="ln_nmr")
                        nc.vector.scalar_tensor_tensor(
                            out=nmr[:], in0=mu[:], scalar=-1.0, in1=rs_[:],
                            op0=ALU.mult, op1=ALU.mult)
                        zf = wk.tile([P, C], F32, tag="ln_z")
                        nc.scalar.activation(zf[:], xt[:], AF.Identity, bias=nmr[:], scale=rs_[:])
                        nc.vector.tensor_mul(zf[:], zf[:], gr[:])
                        zb = wk.tile([P, C], BF16, tag="ln_zb")
                        nc.vector.tensor_add(zb[:], zf[:], br[:])
                        for c4 in range(C // P):
                            tp = ptp.tile([P, P], BF16, tag="tp")
                            nc.tensor.transpose(tp[:], zb[:, c4 * P:(c4 + 1) * P], ident[:])
                            nc.vector.tensor_copy(zt_sbuf[:, c4, m * P:(m + 1) * P], tp[:])

            # ============ P1: LN1 -> zT ============
            if "ln1" not in _SKIP:
                ln_phase(x_t, ln_rep.ap()[0], ln_rep.ap()[1])

            # ============ P2: QKV ============
            if "p2" not in _SKIP:
             with tc.tile_pool(name="p2", bufs=2) as wk, \
                 tc.tile_pool(name="p2c", bufs=1) as cst, \
                 tc.tile_pool(name="p2ps", bufs=2, space="PSUM") as pps:
                wqkv_s = cst.tile([P, C // P, 3 * C], BF16, name="wqkv")
                nc.gpsimd.dma_start(wqkv_s[:], w_qkv.ap().rearrange("(ko p) n -> p ko n", p=P))
                bqkv_s = cst.tile([P, 3 * C], F32, name="bqkv")
                nc.sync.dma_start(bqkv_s[:], b_qkv.ap())
                kv_sh_t = kv_shard[:].rearrange("(m p) n -> p m n", p=P)
                for m in range(NW):
                    for nb in range(3):
                        ps = pps.tile([P, 512], F32, tag="ps")
                        for ko in range(C // P):
                            nc.tensor.matmul(
                                ps[:],
                                lhsT=zt_sbuf[:, ko, m * P:(m + 1) * P],
                                rhs=wqkv_s[:, ko, nb * 512:(nb + 1) * 512],
                                start=(ko == 0), stop=(ko == C // P - 1))
                        if nb == 0:
                            nc.vector.scalar_tensor_tensor(
                                out=q_sbuf[:, m, :], in0=ps[:], scalar=1.0,
                                in1=bqkv_s[:, 0:512], op0=ALU.mult, op1=ALU.add)
                        else:
                            kvb = wk.tile([P, 512], BF16, tag="kvb")
                            nc.vector.scalar_tensor_tensor(
                                out=kvb[:], in0=ps[:], scalar=1.0,
                                in1=bqkv_s[:, nb * 512:(nb + 1) * 512],
                                op0=ALU.mult, op1=ALU.add)
                            nc.sync.dma_start(kv_sh_t[:, m, (nb - 1) * 512:nb * 512], kvb[:])

            # ============ P3: AllGather ============
            if "ag" not in _SKIP:
                nc.gpsimd.collective_compute(
                    "AllGather", ALU.bypass, replica_groups=[list(range(NCORES))],
                    ins=[kv_shard[:].opt()], outs=[kvt[:].opt()])

            # ============ P4: edge windows ============
            if "edge" not in _SKIP:
                with tc.tile_pool(name="big", bufs=2) as big, \
                   tc.tile_pool(name="ew", bufs=3) as wk, \
                   tc.tile_pool(name="pqe", bufs=1, space="PSUM") as pqe, \
                   tc.tile_pool(name="ppsW", bufs=1, space="PSUM") as ppsW, \
                   tc.tile_pool(name="ptp2", bufs=1, space="PSUM") as ptp2:
                  for w in range(NW):
                      idx_t = big.tile([P, TW // 16], I16, tag="idx")
                      nc.gpsimd.dma_start(idx_t[:], eidx.ap()[w])
                      kv_g = big.tile([P, NT, 2 * C], BF16, tag="kv")
                      nc.gpsimd.dma_gather(
                          out_ap=kv_g[:], in_ap=kvt[:], idxs_ap=idx_t[:],
                          num_idxs=TW, num_idxs_reg=TW, elem_size=2 * C,
                          single_packet=False)
                      s1_t = big.tile([P, NT, P], BF16, tag="s1")
                      nc.sync.dma_start(s1_t[:], s_e2r.ap()[w])
                      s2_t = big.tile([P, NT, P], BF16, tag="s2")
                      nc.sync.dma_start(s2_t[:], s_r2e.ap()[w])
                      geo_t = big.tile([P, NT, 8], F32, tag="geo")
                      nc.sync.dma_start(geo_t[:], geo.ap()[w])
                      bias_t = big.tile([P, NT, 8], F32, tag="bias")
                      nc.sync.dma_start(bias_t[:], biasA.ap()[w])

                      psW = ppsW.tile([P, 552], F32, tag="psW")
                      for t0 in range(0, NT, 4):
                          tb = min(4, NT - t0)
                          ke = kv_g[:, t0:t0 + tb, 0:C]
                          ve = kv_g[:, t0:t0 + tb, C:2 * C]
                          qe_ps = pqe.tile([P, 4, C], F32, tag="qe")
                          for d_ in range(tb):
                              nc.tensor.matmul(qe_ps[:, d_, :], lhsT=s2_t[:, t0 + d_, :],
                                               rhs=q_sbuf[:, w, :], start=True, stop=True)
                          diff = wk.tile([P, 4, C], BF16, tag="diff")
                          nc.vector.scalar_tensor_tensor(
                              out=diff[:, 0:tb, :], in0=qe_ps[:, 0:tb, :], scalar=1.0,
                              in1=ke, op0=ALU.mult, op1=ALU.subtract)
                          dsq = wk.tile([P, 4, C], BF16, tag="dsq")
                          nc.scalar.activation(dsq[:, 0:tb, :], diff[:, 0:tb, :], AF.Square)
                          s8 = wk.tile([P, 4, H], F32, tag="s8")
                          nc.vector.reduce_sum(
                              s8[:, 0:tb, :],
                              dsq[:, 0:tb, :].rearrange("p t (h d) -> p t h d", h=H),
                              axis=AX.X)
                          sc = wk.tile([P, 4, H], F32, tag="sc")
                          nc.vector.scalar_tensor_tensor(
                              out=sc[:, 0:tb, :], in0=s8[:, 0:tb, :], scalar=-inv_s,
                              in1=bias_t[:, t0:t0 + tb, :], op0=ALU.mult, op1=ALU.add)
                          aux = wk.tile([P, 4, 40], BF16, tag="aux")
                          nc.scalar.activation(aux[:, 0:tb, 0:8], sc[:, 0:tb, :], AF.Exp)
                          rd = wk.tile([P, 4, 1], F32, tag="rd")
                          nc.vector.reciprocal(rd[:, 0:tb, :], geo_t[:, t0:t0 + tb, 3:4])
                          d4 = wk.tile([P, 4, 4], F32, tag="d4")
                          nc.vector.tensor_sub(d4[:, 0:tb, :], geo_t[:, t0:t0 + tb, 0:4],
                                               geo_t[:, t0:t0 + tb, 4:8])
                          rel = wk.tile([P, 4, 4], F32, tag="rel")
                          nc.vector.tensor_mul(
                              rel[:, 0:tb, :], d4[:, 0:tb, :],
                              rd[:, 0:tb, :].broadcast_to([P, tb, 4]))
                          nc.vector.tensor_mul(
                              aux[:, 0:tb, 8:40].rearrange("p t (h r) -> p t h r", h=H),
                              aux[:, 0:tb, 0:8].unsqueeze(3).broadcast_to([P, tb, H, 4]),
                              rel[:, 0:tb, :].unsqueeze(2).broadcast_to([P, tb, H, 4]))
                          pev = wk.tile([P, 4, C], BF16, tag="pev")
                          nc.vector.tensor_mul(
                              pev[:, 0:tb, :].rearrange("p t (h d) -> p t h d", h=H),
                              aux[:, 0:tb, 0:8].unsqueeze(3).broadcast_to([P, tb, H, DH]),
                              ve.rearrange("p t (h d) -> p t h d", h=H))
                          for d_ in range(tb):
                              t = t0 + d_
                              nc.tensor.matmul(psW[:, 0:512], lhsT=s1_t[:, t, :],
                                               rhs=pev[:, d_, :],
                                               start=(t == 0), stop=(t == NT - 1))
                              nc.tensor.matmul(psW[:, 512:552], lhsT=s1_t[:, t, :],
                                               rhs=aux[:, d_, :],
                                               start=(t == 0), stop=(t == NT - 1))

                      den = wk.tile([P, H], F32, tag="den")
                      nc.vector.tensor_scalar_max(den[:], psW[:, 512:520], 1e-30)
                      rden = wk.tile([P, H], F32, tag="rden")
                      nc.vector.reciprocal(rden[:], den[:])
                      outn = wk.tile([P, C], F32, tag="outn")
                      nc.vector.tensor_mul(
                          outn[:].rearrange("p (h d) -> p h d", h=H),
                          psW[:, 0:512].rearrange("p (h d) -> p h d", h=H),
                          rden[:].unsqueeze(2).broadcast_to([P, H, DH]))
                      an = wk.tile([P, 32], BF16, tag="an")
                      nc.vector.tensor_mul(
                          an[:].rearrange("p (h r) -> p h r", h=H),
                          psW[:, 520:552].rearrange("p (h r) -> p h r", h=H),
                          rden[:].unsqueeze(2).broadcast_to([P, H, 4]))
                      an_tp = ptp2.tile([32, P], BF16, tag="tp2")
                      nc.tensor.transpose(an_tp[:], an[:], ident[:])
                      an_ts = wk.tile([32, P], BF16, tag="an_ts")
                      nc.vector.tensor_copy(an_ts[:], an_tp[:])
                      out2 = ptp2.tile([P, C], F32, tag="out2")
                      nc.tensor.matmul(out2[:], lhsT=an_ts[:], rhs=wvec_s[:],
                                       start=True, stop=True)
                      attin = wk.tile([P, C], BF16, tag="attin")
                      nc.vector.tensor_add(attin[:], outn[:], out2[:])
                      nc.sync.dma_start(
                          attin_d[:].rearrange("(m p) n -> p m n", p=P)[:, w, :], attin[:])

            # ============ P5: x2 = x + attin@Wo + bo ============
            if "p5" not in _SKIP:
             with tc.tile_pool(name="p5", bufs=2) as wk, \
                 tc.tile_pool(name="p5c", bufs=1) as cst, \
                 tc.tile_pool(name="p5tp", bufs=2, space="PSUM") as ptp, \
                 tc.tile_pool(name="p5ps", bufs=2, space="PSUM") as pps:
                wo_s = cst.tile([P, C // P, C], BF16, name="wo")
                nc.gpsimd.dma_start(wo_s[:], w_o.ap().rearrange("(ko p) n -> p ko n", p=P))
                bo_s = cst.tile([P, C], F32, name="bo")
                nc.sync.dma_start(bo_s[:], b_o.ap())
                attin_t = attin_d[:].rearrange("(m p) n -> p m n", p=P)
                x2_t = x2_d[:].rearrange("(m p) n -> p m n", p=P)
                for m in range(NW):
                    at_b = wk.tile([P, C], BF16, tag="at_b")
                    nc.sync.dma_start(at_b[:], attin_t[:, m, :])
                    at_T = wk.tile([P, C // P, P], BF16, tag="at_T")
                    for c4 in range(C // P):
                        tp = ptp.tile([P, P], BF16, tag="tp")
                        nc.tensor.transpose(tp[:], at_b[:, c4 * P:(c4 + 1) * P], ident[:])
                        nc.vector.tensor_copy(at_T[:, c4, :], tp[:])
                    ps = pps.tile([P, C], F32, tag="ps")
                    for ko in range(C // P):
                        nc.tensor.matmul(ps[:], lhsT=at_T[:, ko, :], rhs=wo_s[:, ko, :],
                                         start=(ko == 0), stop=(ko == C // P - 1))
                    xt = wk.tile([P, C], F32, tag="x")
                    nc.sync.dma_start(xt[:], x_t[:, m, :])
                    x2t = wk.tile([P, C], F32, tag="x2")
                    nc.vector.scalar_tensor_tensor(
                        out=x2t[:], in0=ps[:], scalar=1.0, in1=bo_s[:],
                        op0=ALU.mult, op1=ALU.add)
                    nc.vector.tensor_add(x2t[:], x2t[:], xt[:])
                    nc.sync.dma_start(x2_t[:, m, :], x2t[:])

            # ============ P6: LN2 -> zT ============
            if "p6" not in _SKIP:
             ln_phase(x2_d[:].rearrange("(m p) n -> p m n", p=P),
                     ln_rep.ap()[2], ln_rep.ap()[3])

            # ============ P7: FFN1 ============
            if "p7" not in _SKIP:
             with tc.tile_pool(name="p7", bufs=2) as wk, \
                 tc.tile_pool(name="p7c", bufs=1) as cst, \
                 tc.tile_pool(name="p7ps", bufs=2, space="PSUM") as pps:
                w1_s = cst.tile([P, C // P, HID], BF16, name="w1")
                nc.gpsimd.dma_start(w1_s[:], w_1.ap().rearrange("(ko p) n -> p ko n", p=P))
                b1_s = cst.tile([P, HID], F32, name="b1")
                nc.sync.dma_start(b1_s[:], b_1.ap())
                h_t = h_d[:].rearrange("(m p) n -> p m n", p=P)
                for m in range(NW):
                    for nb in range(HID // 512):
                        ps = pps.tile([P, 512], F32, tag="ps")
                        for ko in range(C // P):
                            nc.tensor.matmul(
                                ps[:], lhsT=zt_sbuf[:, ko, m * P:(m + 1) * P],
                                rhs=w1_s[:, ko, nb * 512:(nb + 1) * 512],
                                start=(ko == 0), stop=(ko == C // P - 1))
                        hb = wk.tile([P, 512], F32, tag="hb")
                        nc.vector.scalar_tensor_tensor(
                            out=hb[:], in0=ps[:], scalar=1.0,
                            in1=b1_s[:, nb * 512:(nb + 1) * 512],
                            op0=ALU.mult, op1=ALU.add)
                        hg = wk.tile([P, 512], BF16, tag="hg")
                        nc.scalar.activation(hg[:], hb[:], AF.Gelu_apprx_tanh)
                        nc.sync.dma_start(h_t[:, m, nb * 512:(nb + 1) * 512], hg[:])

            # ============ P8: y = h@W2 + b2 + x2 ============
            if "p8" not in _SKIP:
             with tc.tile_pool(name="p8", bufs=2) as wk, \
                 tc.tile_pool(name="p8c", bufs=1) as cst, \
                 tc.tile_pool(name="p8tp", bufs=2, space="PSUM") as ptp, \
                 tc.tile_pool(name="p8ps", bufs=2, space="PSUM") as pps:
                w2_s = cst.tile([P, HID // P, C], BF16, name="w2")
                nc.gpsimd.dma_start(w2_s[:], w_2.ap().rearrange("(ko p) n -> p ko n", p=P))
                b2_s = cst.tile([P, C], F32, name="b2")
                nc.sync.dma_start(b2_s[:], b_2.ap())
                h_t = h_d[:].rearrange("(m p) n -> p m n", p=P)
                x2_t = x2_d[:].rearrange("(m p) n -> p m n", p=P)
                y_t = y_out.ap().rearrange("(m p) n -> p m n", p=P)
                for m in range(NW):
                    hb = wk.tile([P, HID], BF16, tag="hb")
                    nc.sync.dma_start(hb[:], h_t[:, m, :])
                    h_T = wk.tile([P, HID // P, P], BF16, tag="hT")
                    for c8 in range(HID // P):
                        tp = ptp.tile([P, P], BF16, tag="tp")
                        nc.tensor.transpose(tp[:], hb[:, c8 * P:(c8 + 1) * P], ident[:])
                        nc.vector.tensor_copy(h_T[:, c8, :], tp[:])
                    ps = pps.tile([P, C], F32, tag="ps")
                    for ko in range(HID // P):
                        nc.tensor.matmul(ps[:], lhsT=h_T[:, ko, :], rhs=w2_s[:, ko, :],
                                         start=(ko == 0), stop=(ko == HID // P - 1))
                    x2t = wk.tile([P, C], F32, tag="x2")
                    nc.sync.dma_start(x2t[:], x2_t[:, m, :])
                    yt = wk.tile([P, C], F32, tag="y")
                    nc.vector.scalar_tensor_tensor(
                        out=yt[:], in0=ps[:], scalar=1.0, in1=b2_s[:],
                        op0=ALU.mult, op1=ALU.add)
                    nc.vector.tensor_add(yt[:], yt[:], x2t[:])
                    nc.sync.dma_start(y_t[:, m, :], yt[:])

    nc.compile()
    return nc


def _prep(inputs):
    row = np.asarray(inputs["row_index"]).astype(np.int64).ravel()
    col = np.asarray(inputs["col_index"]).astype(np.int64).ravel()
    tcol = np.asarray(inputs["to_col_index"]).astype(np.int64).ravel()
    bias = np.asarray(inputs["pos_att_bias"], dtype=np.float32)
    dist = np.asarray(inputs["dist"], dtype=np.float32).ravel()
    pos = np.asarray(inputs["pos"], dtype=np.float32)
    cpos = np.asarray(inputs["col_pos"], dtype=np.float32)

    order = np.argsort(row, kind="stable")
    rs_, cs_, ts_ = row[order], col[order], tcol[order]
    win = rs_ // P
    counts = np.bincount(win, minlength=L // P)
    TW = int(np.ceil(max(int(counts.max()), 1) / P) * P)
    NT = TW // P
    starts = np.zeros(L // P + 1, np.int64)
    np.cumsum(counts, out=starts[1:])

    eidx_h = np.zeros((NCORES, NW, P, TW // 16), np.int16)
    geo_h = np.zeros((NCORES, NW, P, NT, 8), np.float32)
    geo_h[..., 3] = 1.0  # pad: dist slot 1 -> rel4 = [0,0,0,1]
    bias_h = np.full((NCORES, NW, P, NT, 8), -1e4, np.float32)
    s1_h = np.zeros((NCORES, NW, P, NT, P), np.float32)
    s2_h = np.zeros((NCORES, NW, P, NT, P), np.float32)

    for gw in range(L // P):
        c, w = divmod(gw, NW)
        s, e = int(starts[gw]), int(starts[gw + 1])
        n = e - s
        if n == 0:
            continue
        ecols = cs_[s:e]
        erows = (rs_[s:e] - gw * P).astype(np.int64)
        eo = order[s:e]
        j = np.arange(n)
        wrap = np.zeros((16, TW // 16), np.int16)
        wrap[j % 16, j // 16] = ecols.astype(np.int16)
        eidx_h[c, w] = np.tile(wrap, (8, 1))
        t_of = j // P
        e_of = j % P
        geo_h[c, w, e_of, t_of, 0:3] = cpos[ts_[s:e]]
        geo_h[c, w, e_of, t_of, 3] = dist[eo]
        geo_h[c, w, e_of, t_of, 4:7] = pos[rs_[s:e]]
        bias_h[c, w, e_of, t_of, :] = bias[eo]
        s1_h[c, w, e_of, t_of, erows] = 1.0
        s2_h[c, w, erows, t_of, e_of] = 1.0

    import ml_dtypes
    return (TW, eidx_h, geo_h, bias_h,
            s1_h.astype(ml_dtypes.bfloat16), s2_h.astype(ml_dtypes.bfloat16))


def kernel(**inputs):
    x = np.asarray(inputs["x"], dtype=np.float32)
    TW, eidx_h, geo_h, bias_h, s1_h, s2_h = _prep(inputs)
    if TW not in _cache:
        _cache[TW] = _build(TW)
    nc = _cache[TW]

    f32 = lambda k: np.asarray(inputs[k], np.float32)
    rep = lambda v: np.ascontiguousarray(np.broadcast_to(v[None, :], (P, v.shape[0])))
    w_qkv = np.concatenate([f32("Wq"), f32("Wk"), f32("Wv")], axis=1)
    b_qkv = rep(np.concatenate([f32("bq"), f32("bk"), f32("bv")]))
    wv4 = np.concatenate([f32("Wvec"), f32("bvec")[None, :]], axis=0)
    w_vec4 = np.zeros((32, C), np.float32)
    for h in range(H):
        w_vec4[4 * h:4 * h + 4, h * DH:(h + 1) * DH] = wv4[:, h * DH:(h + 1) * DH]
    ln_rep = np.stack([rep(f32("ln1_g")), rep(f32("ln1_b")),
                       rep(f32("ln2_g")), rep(f32("ln2_b"))])

    in_maps = []
    for c in range(NCORES):
        in_maps.append(dict(
            x_in=np.ascontiguousarray(x[c * RS:(c + 1) * RS]),
            w_qkv=w_qkv, b_qkv=b_qkv,
            w_o=f32("Wo"), b_o=rep(f32("bo")),
            w_1=f32("W1"), b_1=rep(f32("b1")),
            w_2=f32("W2"), b_2=rep(f32("b2")),
            w_vec4=w_vec4, ln_rep=ln_rep, ident_in=np.eye(P, dtype=np.float32),
            eps_in=np.full((P, 1), 1e-5, np.float32),
            eidx=eidx_h[c], geo=geo_h[c], biasA=bias_h[c],
            s_e2r=s1_h[c], s_r2e=s2_h[c],
        ))
    _last["nc"] = nc
    _last["in_maps"] = in_maps
    res = run_bass_kernel_spmd(nc, in_maps, list(range(NCORES)))
    y = np.concatenate([res.results[c]["y_out"] for c in range(NCORES)], axis=0)
    return np.asarray(y, np.float32)


_last = {}



# revision 28
# speedup vs baseline: 1.5225x; 1.5225x over previous
"""Trainium2 Bass kernel for nn_EncoderLayer_88476326298146 (sparse graph attention).

Row-sharded across 8 NeuronCores: core c owns nodes [c*2048, (c+1)*2048) and the
edges targeting them (host-sorted by row into 16 windows of 128 rows, padded to a
fixed per-window count TW). Engine-balanced v2:
 - LN affine + all linear biases folded into weights host-side (ones-row matmuls
   add remaining bias rows on the PE).
 - k is stored NEGATED in the AllGathered kv table, so the per-edge difference
   qe-ke is accumulated on the PE (scatter matmul + identity matmul), squared on
   the Activation engine, head-reduced on DVE.
 - pexp*v runs mostly on the (otherwise idle) GPSIMD engine.
 - The positional-attention term an@Wvec@Wo is folded into an extended 640-col
   attention row ([outn | an | pad]) with a host-precomputed [Wo; Wvec@Wo; 0].
 - FFN1 is computed transposed (lhsT=W1 natural layout) so the gelu bias is a
   per-partition Activation bias and FFN2 needs no transposes.
 - Transposes ride the DMA xbar (dma_start_transpose, one op per row-window).
 - AllGather is split into 4 chunks overlapped with the QKV windows.
All intermediates (x, zT, q, x2, att) stay SBUF-resident.
"""
import os
import numpy as np

import concourse.bass as bass
import concourse.bacc as bacc
import concourse.mybir as mybir
import concourse.tile as tile
from concourse.bass_utils import run_bass_kernel_spmd
from concourse.library_config import mlp as mlp_lib

L, E, SP, C, H, DH, HID = 16384, 131072, 20000, 512, 8, 64, 1024
NCORES = 8
RS = L // NCORES
NW = RS // 128
P = 128
KO = C // P          # 4 c-chunks
HC = HID // P        # 8 hid-chunks
NAG = 1              # AllGather chunks (fake_nrt allows one writer per Shared tensor)
AGW = NW // NAG      # windows per AG chunk
F32 = mybir.dt.float32
BF16 = mybir.dt.bfloat16
I16 = mybir.dt.int16
AF = mybir.ActivationFunctionType
ALU = mybir.AluOpType
AX = mybir.AxisListType

_cache = {}
_SKIP = set(os.environ.get("KSKIP", "").split(","))
POOL_PEV = 4   # heads 0:POOL_PEV of each pev go to gpsimd, the rest to DVE


def _build(TW, NTW):
    NT = TW // P
    nc = bacc.Bacc("TRN2", target_bir_lowering=False, debug=False, num_devices=NCORES)

    x_in = nc.dram_tensor("x_in", [RS, C], BF16, kind="ExternalInput")
    w_qkv = nc.dram_tensor("w_qkv", [C, 3 * C], BF16, kind="ExternalInput")
    b_qkv = nc.dram_tensor("b_qkv", [1, 3 * C], BF16, kind="ExternalInput")
    w_o5 = nc.dram_tensor("w_o5", [640, C], BF16, kind="ExternalInput")
    b_o = nc.dram_tensor("b_o", [1, C], BF16, kind="ExternalInput")
    w_1 = nc.dram_tensor("w_1", [C, HID], BF16, kind="ExternalInput")
    b_1r = nc.dram_tensor("b_1r", [1, HID], BF16, kind="ExternalInput")
    w_2 = nc.dram_tensor("w_2", [HID, C], BF16, kind="ExternalInput")
    b_2 = nc.dram_tensor("b_2", [1, C], BF16, kind="ExternalInput")
    ident_in = nc.dram_tensor("ident_in", [P, P], BF16, kind="ExternalInput")
    eps_in = nc.dram_tensor("eps_in", [P, 1], F32, kind="ExternalInput")
    eidx = nc.dram_tensor("eidx", [NW, P, TW // 16], I16, kind="ExternalInput")
    geob = nc.dram_tensor("geob", [NW, P, NT, 16], F32, kind="ExternalInput")
    s_12 = nc.dram_tensor("s_12", [NW, P, NT, 2 * P], BF16, kind="ExternalInput")
    y_out = nc.dram_tensor("y_out", [RS, C], F32, kind="ExternalOutput")

    x_t = x_in.ap().rearrange("(m p) n -> p m n", p=P)
    y_t = y_out.ap().rearrange("(m p) n -> p m n", p=P)
    inv_s = 1.0 / float(np.sqrt(DH))

    with tile.TileContext(nc) as tc:
        with tc.tile_pool(name="dram", bufs=1, space="DRAM") as dram, \
             tc.tile_pool(name="const", bufs=1) as const, \
             tc.tile_pool(name="state", bufs=1) as state:
            nc.gpsimd.load_library(mlp_lib)

            ident = const.tile([P, P], BF16)
            nc.sync.dma_start(ident[:], ident_in.ap())
            eps_t = const.tile([P, 1], F32)
            nc.sync.dma_start(eps_t[:], eps_in.ap())
            ones1 = const.tile([1, P], BF16)
            nc.vector.memset(ones1[:], 1.0)
            wqkv_s = const.tile([P, KO, 3 * C], BF16, name="wqkv")
            nc.sync.dma_start(wqkv_s[:], w_qkv.ap().rearrange("(k p) n -> p k n", p=P))
            bqkv_r = const.tile([1, 3 * C], BF16, name="bqkv")
            nc.sync.dma_start(bqkv_r[:], b_qkv.ap())
            wo5_s = const.tile([P, 5, C], BF16, name="wo5")
            nc.sync.dma_start(wo5_s[:], w_o5.ap().rearrange("(k p) n -> p k n", p=P))
            bo_r = const.tile([1, C], BF16, name="bo")
            nc.sync.dma_start(bo_r[:], b_o.ap())
            w1_s = const.tile([P, KO, HID], BF16, name="w1")
            nc.sync.dma_start(w1_s[:], w_1.ap().rearrange("(k p) n -> p k n", p=P))
            b1_r = const.tile([1, HID], BF16, name="b1r")
            nc.sync.dma_start(b1_r[:], b_1r.ap())
            w2_s = const.tile([P, HC, C], BF16, name="w2")
            nc.sync.dma_start(w2_s[:], w_2.ap().rearrange("(k p) n -> p k n", p=P))
            b2_r = const.tile([1, C], BF16, name="b2")
            nc.sync.dma_start(b2_r[:], b_2.ap())

            x_sb = state.tile([P, NW, C], BF16, name="x_sb")
            zt_sb = state.tile([P, KO, RS], BF16, name="zt_sb")
            q_sb = state.tile([P, NW, C], BF16, name="q_sb")
            x2_sb = state.tile([P, NW, C], BF16, name="x2_sb")

            kv_shard = dram.tile([RS, 2 * C], BF16)
            if "ag" not in _SKIP:
                kvt = dram.tile([L, 2 * C], BF16, addr_space="Shared")
            else:
                kvt = dram.tile([L, 2 * C], BF16)
            kv_sh_t = kv_shard[:].rearrange("(m p) n -> p m n", p=P)

            # Newton rsqrt on DVE (avoids Act Sqrt -> no act-table thrash).
            # Works on [P, n] f32 tiles; 2 Newton steps from the bit-trick seed.
            def rsqrt_newton(v, wk, pref):
                n = v.shape[1]
                ve = wk.tile([P, n], F32, tag=pref + "_ve", name="ve")
                nc.vector.tensor_scalar_add(ve[:], v[:], 1e-5)
                rv = wk.tile([P, n], F32, tag=pref + "_rv", name="rv")
                nc.vector.reciprocal(rv[:], ve[:])
                y = wk.tile([P, n], F32, tag=pref + "_y", name="y")
                nc.vector.tensor_scalar(
                    out=y[:], in0=rv[:], scalar1=0.5, scalar2=0.5,
                    op0=ALU.mult, op1=ALU.add)
                t = wk.tile([P, n], F32, tag=pref + "_t", name="t")
                for _ in range(3):
                    nc.vector.tensor_mul(t[:], y[:], y[:])
                    nc.vector.tensor_mul(t[:], t[:], ve[:])
                    nc.vector.tensor_scalar(
                        out=t[:], in0=t[:], scalar1=-0.5, scalar2=1.5,
                        op0=ALU.mult, op1=ALU.add)
                    nc.vector.tensor_mul(y[:], y[:], t[:])
                return y

            # per-window LN stats + normalize: src slice [P, C] -> zb bf16
            def ln_win(src, wk, zb):
                sx = wk.tile([P, 1], F32, tag="ln_sx")
                nc.vector.reduce_sum(sx[:], src, axis=AX.X)
                sq = wk.tile([P, C], BF16, tag="ln_sq")
                sx2 = wk.tile([P, 1], F32, tag="ln_sx2")
                nc.vector.scalar_tensor_tensor(
                    out=sq[:], in0=src, scalar=1.0, in1=src,
                    op0=ALU.mult, op1=ALU.mult, accum_out=sx2[:])
                mu = wk.tile([P, 1], F32, tag="ln_mu")
                nc.vector.tensor_scalar_mul(mu[:], sx[:], 1.0 / C)
                mu2 = wk.tile([P, 1], F32, tag="ln_mu2")
                nc.vector.tensor_mul(mu2[:], mu[:], mu[:])
                var = wk.tile([P, 1], F32, tag="ln_var")
                nc.vector.scalar_tensor_tensor(
                    out=var[:], in0=sx2[:], scalar=1.0 / C, in1=mu2[:],
                    op0=ALU.mult, op1=ALU.subtract)
                rs_ = rsqrt_newton(var, wk, "ln")
                nc.vector.tensor_scalar(
                    out=zb[:], in0=src, scalar1=mu[:], scalar2=rs_[:],
                    op0=ALU.subtract, op1=ALU.mult)

            # ============ HEAD: load x, LN1 block, then dense QKV + chunked AG ============
            nc.sync.dma_start(x_sb[:], x_t)
            if "head" not in _SKIP:
             with tc.tile_pool(name="hd", bufs=3) as wk, \
                 tc.tile_pool(name="hdst", bufs=1) as hst, \
                 tc.tile_pool(name="hdps", bufs=3, space="PSUM") as pps:
                sxA = hst.tile([P, NW], F32, name="sxA")
                sx2A = hst.tile([P, NW], F32, name="sx2A")
                for m in range(NW):
                    nc.vector.reduce_sum(sxA[:, m:m + 1], x_sb[:, m, :], axis=AX.X)
                    sq = wk.tile([P, C], BF16, tag="ln_sq")
                    nc.vector.scalar_tensor_tensor(
                        out=sq[:], in0=x_sb[:, m, :], scalar=1.0, in1=x_sb[:, m, :],
                        op0=ALU.mult, op1=ALU.mult, accum_out=sx2A[:, m:m + 1])
                muA = hst.tile([P, NW], F32, name="muA")
                nc.vector.tensor_scalar_mul(muA[:], sxA[:], 1.0 / C)
                mu2A = hst.tile([P, NW], F32, name="mu2A")
                nc.vector.tensor_mul(mu2A[:], muA[:], muA[:])
                varA = hst.tile([P, NW], F32, name="varA")
                nc.vector.scalar_tensor_tensor(
                    out=varA[:], in0=sx2A[:], scalar=1.0 / C, in1=mu2A[:],
                    op0=ALU.mult, op1=ALU.subtract)
                rsA = rsqrt_newton(varA, hst, "h")
                for m in range(NW):
                    zb = wk.tile([P, C], BF16, tag="zb")
                    nc.vector.tensor_scalar(
                        out=zb[:], in0=x_sb[:, m, :], scalar1=muA[:, m:m + 1],
                        scalar2=rsA[:, m:m + 1], op0=ALU.subtract, op1=ALU.mult)
                    nc.sync.dma_start_transpose(
                        zt_sb[:, :, m * P:(m + 1) * P], zb[:])
                for m in range(NW):
                    kvb = wk.tile([P, 2 * C], BF16, tag="kvb")
                    for nb in range(3):
                        ps = pps.tile([P, 512], F32, tag="ps")
                        for ko in range(KO):
                            nc.tensor.matmul(
                                ps[:],
                                lhsT=zt_sb[:, ko, m * P:(m + 1) * P],
                                rhs=wqkv_s[:, ko, nb * 512:(nb + 1) * 512],
                                start=(ko == 0), stop=False)
                        nc.tensor.matmul(
                            ps[:], lhsT=ones1[:], rhs=bqkv_r[:, nb * 512:(nb + 1) * 512],
                            start=False, stop=True)
                        if nb == 0:
                            nc.scalar.activation(q_sb[:, m, :], ps[:], AF.Copy)
                        else:
                            nc.vector.tensor_copy(kvb[:, (nb - 1) * 512:nb * 512], ps[:])
                    nc.gpsimd.dma_start(kv_sh_t[:, m, :], kvb[:])
                    if "ag" not in _SKIP and (m + 1) % AGW == 0:
                        k = m // AGW
                        nc.gpsimd.collective_compute(
                            "AllGather", ALU.bypass,
                            replica_groups=[list(range(NCORES))],
                            ins=[kv_shard[k * AGW * P:(k + 1) * AGW * P, :].opt()],
                            outs=[kvt[k * AGW * P * NCORES:(k + 1) * AGW * P * NCORES, :].opt()])

            # ============ EDGE + TAIL, PE-block schedule ============
            # Block w emits one long PE stream: qe(w) | psW(w-1) | Wo(w-2) |
            # FFN(w-3).  All cross-engine chains (square/reduce/exp/pev, LN2)
            # span block boundaries, so the PE rarely stalls mid-block and
            # holds its ramped clock.
            if "edge" not in _SKIP:
                with tc.tile_pool(name="bigkv", bufs=2) as bigkv, \
                   tc.tile_pool(name="bigs", bufs=3) as bigs, \
                   tc.tile_pool(name="ew", bufs=2) as wk, \
                   tc.tile_pool(name="ewg", bufs=2) as wg, \
                   tc.tile_pool(name="ewp", bufs=10) as wp, \
                   tc.tile_pool(name="tl", bufs=2) as twk, \
                   tc.tile_pool(name="tlh", bufs=1) as twh, \
                   tc.tile_pool(name="tlat", bufs=2) as wat, \
                   tc.tile_pool(name="pqe", bufs=4, space="PSUM") as pqe, \
                   tc.tile_pool(name="ppsW", bufs=1, space="PSUM") as ppsW, \
                   tc.tile_pool(name="tlpxy", bufs=1, space="PSUM") as pxy, \
                   tc.tile_pool(name="tlph", bufs=1, space="PSUM") as pph:
                  wins = [None] * NW
                  pend = [None] * NW     # per-window list of (pev, aux8, aux32)
                  at_Ts = [None] * NW

                  def edge_loads(w):
                      nt = NTW[w]
                      idx_t = bigkv.tile([P, TW // 16], I16, tag="idx")
                      nc.sync.dma_start(idx_t[:, 0:nt * 8], eidx.ap()[w, :, 0:nt * 8])
                      kv_g = bigkv.tile([P, NT, 2 * C], BF16, tag="kv")
                      nc.gpsimd.dma_gather(
                          out_ap=kv_g[:, 0:nt, :], in_ap=kvt[:], idxs_ap=idx_t[:],
                          num_idxs=nt * P, num_idxs_reg=nt * P, elem_size=2 * C,
                          single_packet=False)
                      s12_t = bigs.tile([P, NT, 2 * P], BF16, tag="s12")
                      nc.sync.dma_start(s12_t[:, 0:nt], s_12.ap()[w, :, 0:nt])
                      gb_t = bigkv.tile([P, NT, 16], F32, tag="geob")
                      nc.sync.dma_start(gb_t[:, 0:nt], geob.ap()[w, :, 0:nt])
                      rd = wk.tile([P, NT, 1], F32, tag="rd")
                      nc.vector.reciprocal(rd[:, 0:nt], gb_t[:, 0:nt, 3:4])
                      d4 = wk.tile([P, NT, 4], F32, tag="d4")
                      nc.vector.tensor_sub(d4[:, 0:nt], gb_t[:, 0:nt, 0:4],
                                           gb_t[:, 0:nt, 4:8])
                      rel = wk.tile([P, NT, 4], F32, tag="rel")
                      nc.vector.tensor_mul(rel[:, 0:nt], d4[:, 0:nt],
                                           rd[:, 0:nt].broadcast_to([P, nt, 4]))
                      return dict(kv=kv_g, s12=s12_t, gb=gb_t, rel=rel)

                  def mid_pair(w, t0, pi, qeA, qeB):
                      d = wins[w]
                      tb = 2 if qeB is not None else 1
                      dsq = wg.tile([P, 2, C], BF16, tag="dsq")
                      nc.scalar.activation(dsq[:, 0, :], qeA[:], AF.Square)
                      if qeB is not None:
                          nc.scalar.activation(dsq[:, 1, :], qeB[:], AF.Square)
                      pr = wg.tile([P, 2, H, DH // 2], BF16, tag="pr")
                      dsq4 = dsq[:].rearrange("p t (h d) -> p t h d", h=H)
                      nc.vector.tensor_add(pr[:, 0:tb], dsq4[:, 0:tb, :, 0:DH // 2],
                                           dsq4[:, 0:tb, :, DH // 2:DH])
                      s8 = wg.tile([P, 2, H], F32, tag="s8")
                      nc.vector.reduce_sum(s8[:, 0:tb], pr[:, 0:tb], axis=AX.X)
                      sc = wg.tile([P, 2, H], F32, tag="sc")
                      nc.vector.scalar_tensor_tensor(
                          out=sc[:, 0:tb], in0=s8[:, 0:tb], scalar=-inv_s,
                          in1=d["gb"][:, t0:t0 + tb, 8:16], op0=ALU.mult, op1=ALU.add)
                      aux8 = wp.tile([P, 2, 8], BF16, tag="aux8")
                      nc.scalar.activation(aux8[:, 0:tb], sc[:, 0:tb], AF.Exp)
                      aux32 = wp.tile([P, 2, 32], BF16, tag="aux32")
                      nc.vector.tensor_mul(
                          aux32[:, 0:tb].rearrange("p t (h r) -> p t h r", h=H),
                          aux8[:, 0:tb].unsqueeze(3).broadcast_to([P, tb, H, 4]),
                          d["rel"][:, t0:t0 + tb].unsqueeze(2).broadcast_to([P, tb, H, 4]))
                      pev = wp.tile([P, 2, C], BF16, tag="pev")
                      eng = nc.gpsimd if pi < POOL_PEV else nc.vector
                      eng.tensor_mul(
                          pev[:, 0:tb].rearrange("p t (h d) -> p t h d", h=H),
                          aux8[:, 0:tb].unsqueeze(3).broadcast_to([P, tb, H, DH]),
                          d["kv"][:, t0:t0 + tb, C:2 * C].rearrange(
                              "p t (h d) -> p t h d", h=H))
                      return pev, aux8, aux32

                  def sect_qe(w):
                      d = wins[w]
                      nt = NTW[w]
                      out = []
                      hold = []
                      t0 = 0
                      for t in range(nt):
                          qe_ps = pqe.tile([P, C], F32, tag="qe")
                          nc.tensor.matmul(qe_ps[:], lhsT=d["s12"][:, t, P:2 * P],
                                           rhs=q_sb[:, w, :], start=True, stop=False)
                          nc.tensor.matmul(qe_ps[:], lhsT=ident[:],
                                           rhs=d["kv"][:, t, 0:C], start=False, stop=True)
                          hold.append(qe_ps)
                          if len(hold) == 2 or t == nt - 1:
                              qeA = hold[0]
                              qeB = hold[1] if len(hold) == 2 else None
                              out.append(mid_pair(w, t0, len(out), qeA, qeB))
                              hold = []
                              t0 = t + 1
                      return out

                  def sect_psw(w):
                      d = wins[w]
                      nt = NTW[w]
                      psW = ppsW.tile([P, 552], F32, tag="psW", name="psW")
                      for t in range(nt):
                          pev, auxc = pend[w][t // 2]
                          d_ = t % 2
                          nc.tensor.matmul(psW[:, 0:512], lhsT=d["s12"][:, t, 0:P],
                                           rhs=pev[:, d_, :],
                                           start=(t == 0), stop=(t == nt - 1))
                          nc.tensor.matmul(psW[:, 512:552], lhsT=d["s12"][:, t, 0:P],
                                           rhs=auxc[:, d_, :],
                                           start=(t == 0), stop=(t == nt - 1))
                      den = wk.tile([P, H], F32, tag="den")
                      nc.vector.tensor_scalar_max(den[:], psW[:, 512:520], 1e-30)
                      rden = wk.tile([P, H], F32, tag="rden")
                      nc.vector.reciprocal(rden[:], den[:])
                      attx = wk.tile([P, 640], BF16, tag="attx")
                      nc.vector.memset(attx[:, 544:640], 0.0)
                      nc.vector.tensor_mul(
                          attx[:, 0:512].rearrange("p (h d) -> p h d", h=H),
                          psW[:, 0:512].rearrange("p (h d) -> p h d", h=H),
                          rden[:].unsqueeze(2).broadcast_to([P, H, DH]))
                      nc.vector.tensor_mul(
                          attx[:, 512:544].rearrange("p (h r) -> p h r", h=H),
                          psW[:, 520:552].rearrange("p (h r) -> p h r", h=H),
                          rden[:].unsqueeze(2).broadcast_to([P, H, 4]))
                      at_T = wat.tile([P, 5, P], BF16, tag="at_T")
                      nc.sync.dma_start_transpose(at_T[:], attx[:])
                      pend[w] = None
                      wins[w] = None
                      return at_T

                  def sect_p5(m, at_T):
                      ps = pxy.tile([P, C], F32, tag="pxy", name="ps5")
                      for ko in range(5):
                          nc.tensor.matmul(ps[:], lhsT=at_T[:, ko, :], rhs=wo5_s[:, ko, :],
                                           start=(ko == 0), stop=False)
                      nc.tensor.matmul(ps[:], lhsT=ones1[:], rhs=bo_r[:],
                                       start=False, stop=True)
                      nc.vector.tensor_add(x2_sb[:, m, :], ps[:], x_sb[:, m, :])
                      zb = twk.tile([P, C], BF16, tag="zb2")
                      ln_win(x2_sb[:, m, :], twk, zb)
                      nc.sync.dma_start_transpose(
                          zt_sb[:, :, m * P:(m + 1) * P], zb[:])

                  def sect_ffn(m):
                      hT = twh.tile([P, HC, P], BF16, tag="hT", name="hT")
                      for h4 in range(HC // 4):
                          psh = pph.tile([P, 4, P], F32, tag="psh")
                          for i in range(4):
                              hc = h4 * 4 + i
                              for ko in range(KO):
                                  nc.tensor.matmul(
                                      psh[:, i, :], lhsT=w1_s[:, ko, hc * P:(hc + 1) * P],
                                      rhs=zt_sb[:, ko, m * P:(m + 1) * P],
                                      start=(ko == 0), stop=False)
                              nc.tensor.matmul(
                                  psh[:, i, :], lhsT=b1_r[:, hc * P:(hc + 1) * P],
                                  rhs=ones1[:], start=False, stop=True)
                          nc.scalar.activation(
                              hT[:, h4 * 4:(h4 + 1) * 4, :], psh[:], AF.Gelu_apprx_tanh)
                      psy = pxy.tile([P, C], F32, tag="pxy", name="psy")
                      for hc in range(HC):
                          nc.tensor.matmul(psy[:], lhsT=hT[:, hc, :], rhs=w2_s[:, hc, :],
                                           start=(hc == 0), stop=False)
                      nc.tensor.matmul(psy[:], lhsT=ones1[:], rhs=b2_r[:],
                                       start=False, stop=True)
                      yt = twk.tile([P, C], F32, tag="y")
                      nc.vector.tensor_add(yt[:], psy[:], x2_sb[:, m, :])
                      nc.gpsimd.dma_start(y_t[:, m, :], yt[:])

                  wins[0] = edge_loads(0)
                  for w in range(NW + 3):
                      if w < NW:
                          if w + 1 < NW:
                              wins[w + 1] = edge_loads(w + 1)
                          pend[w] = sect_qe(w)
                      if w - 1 >= 0 and w - 1 < NW:
                          at_Ts[w - 1] = sect_psw(w - 1)
                      if w - 2 >= 0 and w - 2 < NW:
                          sect_p5(w - 2, at_Ts[w - 2])
                          at_Ts[w - 2] = None
                      if w - 3 >= 0 and w - 3 < NW:
                          sect_ffn(w - 3)

    nc.compile()
    return nc


def _prep(inputs):
    import ml_dtypes
    row = np.asarray(inputs["row_index"]).astype(np.int64).ravel()
    col = np.asarray(inputs["col_index"]).astype(np.int64).ravel()
    tcol = np.asarray(inputs["to_col_index"]).astype(np.int64).ravel()
    bias = np.asarray(inputs["pos_att_bias"], dtype=np.float32)
    dist = np.asarray(inputs["dist"], dtype=np.float32).ravel()
    pos = np.asarray(inputs["pos"], dtype=np.float32)
    cpos = np.asarray(inputs["col_pos"], dtype=np.float32)

    order = np.argsort(row, kind="stable")
    rs_, cs_, ts_ = row[order], col[order], tcol[order]
    win = rs_ // P
    counts = np.bincount(win, minlength=L // P)
    TW = int(np.ceil(max(int(counts.max()), 1) / P) * P)
    NT = TW // P
    ntw = np.maximum(1, np.ceil(counts / P).astype(np.int64)).reshape(NCORES, NW)
    starts = np.zeros(L // P + 1, np.int64)
    np.cumsum(counts, out=starts[1:])

    # kvt row order after chunked AllGather: global row (core c, local r) lands
    # at (r//(RS//NAG))*(RS//NAG*NCORES) + c*(RS//NAG) + r%(RS//NAG)
    CH = RS // NAG
    g = np.arange(L)
    cc, rr = g // RS, g % RS
    perm = (rr // CH) * (CH * NCORES) + cc * CH + (rr % CH)

    eidx_h = np.zeros((NCORES, NW, P, TW // 16), np.int16)
    geob_h = np.zeros((NCORES, NW, P, NT, 16), np.float32)
    geob_h[..., 3] = 1.0  # pad: dist slot 1 -> rel4 = [0,0,0,1]
    geob_h[..., 8:16] = -1e4  # pad: bias -> exp ~ 0
    s12_h = np.zeros((NCORES, NW, P, NT, 2 * P), np.float32)

    for gw in range(L // P):
        c, w = divmod(gw, NW)
        s, e = int(starts[gw]), int(starts[gw + 1])
        n = e - s
        if n == 0:
            continue
        ecols = perm[cs_[s:e]]
        erows = (rs_[s:e] - gw * P).astype(np.int64)
        eo = order[s:e]
        j = np.arange(n)
        wrap = np.zeros((16, TW // 16), np.int16)
        wrap[j % 16, j // 16] = ecols.astype(np.int16)
        eidx_h[c, w] = np.tile(wrap, (8, 1))
        t_of = j // P
        e_of = j % P
        geob_h[c, w, e_of, t_of, 0:3] = cpos[ts_[s:e]]
        geob_h[c, w, e_of, t_of, 3] = dist[eo]
        geob_h[c, w, e_of, t_of, 4:7] = pos[rs_[s:e]]
        geob_h[c, w, e_of, t_of, 8:16] = bias[eo]
        s12_h[c, w, e_of, t_of, erows] = 1.0       # s1: edge -> row scatter
        s12_h[c, w, erows, t_of, P + e_of] = 1.0   # s2: row -> edge scatter
    return (TW, ntw, eidx_h, geob_h, s12_h.astype(ml_dtypes.bfloat16))


def kernel(**inputs):
    import ml_dtypes
    BF = ml_dtypes.bfloat16
    x = np.asarray(inputs["x"], dtype=np.float32)
    TW, ntw, eidx_h, geob_h, s12_h = _prep(inputs)
    ntmax = tuple(ntw.max(axis=0))  # same program on all cores: per-window max
    key = (TW, ntmax)
    if key not in _cache:
        _cache[key] = _build(TW, list(ntmax))
    nc = _cache[key]

    f32 = lambda k: np.asarray(inputs[k], np.float32)
    g1 = f32("ln1_g")[:, None]
    b1v = f32("ln1_b")
    g2 = f32("ln2_g")[:, None]
    b2v = f32("ln2_b")
    Wq, Wk, Wv = f32("Wq"), f32("Wk"), f32("Wv")
    # LN1 affine folded into QKV weights; k negated so PE accumulates qe-ke.
    w_qkv = np.concatenate([g1 * Wq, -(g1 * Wk), g1 * Wv], axis=1)
    b_qkv = np.concatenate([b1v @ Wq + f32("bq"), -(b1v @ Wk + f32("bk")),
                            b1v @ Wv + f32("bv")])[None, :]
    W1 = f32("W1")
    w_1 = g2 * W1
    b_1r = (b2v @ W1 + f32("b1"))[None, :]
    wv4 = np.concatenate([f32("Wvec"), f32("bvec")[None, :]], axis=0)
    w_vec4 = np.zeros((32, C), np.float32)
    for h in range(H):
        w_vec4[4 * h:4 * h + 4, h * DH:(h + 1) * DH] = wv4[:, h * DH:(h + 1) * DH]
    Wo = f32("Wo")
    w_o5 = np.zeros((640, C), np.float32)
    w_o5[0:512] = Wo
    w_o5[512:544] = w_vec4 @ Wo

    in_maps = []
    for c in range(NCORES):
        in_maps.append(dict(
            x_in=np.ascontiguousarray(x[c * RS:(c + 1) * RS]).astype(BF),
            w_qkv=w_qkv.astype(BF), b_qkv=b_qkv.astype(BF),
            w_o5=w_o5.astype(BF), b_o=f32("bo")[None, :].astype(BF),
            w_1=w_1.astype(BF), b_1r=b_1r.astype(BF),
            w_2=f32("W2").astype(BF), b_2=f32("b2")[None, :].astype(BF),
            ident_in=np.eye(P, dtype=np.float32).astype(BF),
            eps_in=np.full((P, 1), 1e-5, np.float32),
            eidx=eidx_h[c], geob=geob_h[c], s_12=s12_h[c],
        ))
    _last["nc"] = nc
    _last["in_maps"] = in_maps
    res = run_bass_kernel_spmd(nc, in_maps, list(range(NCORES)))
    y = np.concatenate([res.results[c]["y_out"] for c in range(NCORES)], axis=0)
    return np.asarray(y, np.float32)


_last = {}
